# revision 43
# baseline (speedup 1.0000x reference)
"""BiMambaBlock Trainium2 kernel (8-core SPMD via Bass/Tile), v3.

Launch1 core = (b, dir, s) with s = half of d_inner; the kernel is split
into two time-halves (h0, h1) with instruction emission interleaved so
the PE-heavy in_proj of h1 overlaps the DVE-bound selective scan of h0.
The 16 scan states are packed as 8x512 segments into [128, 2048] mega
tiles (4 n-blocks); cross-half scan state is carried via a dBx
boundary-column fixup with zeroed dA segment starts.

v3 notes (measured on HW): all big elementwise streams live on DVE
(bf16 packed -> 2x mode; GpSimd shares SBUF ports with DVE and its TT
is 3.3x slower per element, so offloading there slows the scan);
activations stay within the ACT exp_and_others table set (tanh-trick
silu, no sigmoid) to avoid ACT_TABLE_LOAD thrash; persistent constants
are loaded via a handful of batched rearranged DMAs (sync-queue issue
cost ~0.6us each dominates small transfers); x ships as bf16; the h1
out_proj is split kt 0..5 / 6..7 so only a short tail trails the last
scan.

Launch2 core = (b, m-half, L-half) gated combine; host permutes rows so
every core's own m-half sits in block 0 (uniform SPMD program).
"""

import os
import sys

sys.path.insert(0, "/opt/trn_rl_repo")

import numpy as np
import ml_dtypes

import concourse.bass as bass
import concourse.mybir as mybir
import concourse.tile as tile
from concourse import bacc
from concourse.bass_utils import run_bass_kernel_spmd

FP32 = mybir.dt.float32
BF16 = mybir.dt.bfloat16
AF = mybir.ActivationFunctionType
OP = mybir.AluOpType
BF = ml_dtypes.bfloat16

B, L, Dm, Di, N, R, KC = 2, 1024, 1024, 2048, 16, 64, 4
DiS = Di // 2
EPS = 1e-5
NCORES = 8
HL = L // 2          # 512 per half
SEG = 4              # segments per n-block
MW = SEG * HL        # 2048 mega width
NXP = Di // 128      # 16
NSH = DiS // 128     # 8
NMD = Dm // 128      # 8


# ----------------------------------------------------------------- launch 1
def build_launch1():
    nc = bacc.Bacc("TRN2", target_bir_lowering=False, debug=False,
                   num_devices=NCORES)
    xT = nc.dram_tensor("xT", [Dm, L], BF16, kind="ExternalInput")
    rstd_in = nc.dram_tensor("rstd_in", [128, L], BF16, kind="ExternalInput")
    mr_in = nc.dram_tensor("mr_in", [1, L], BF16, kind="ExternalInput")
    w_in = nc.dram_tensor("w_in", [Dm, 3072], BF16, kind="ExternalInput")
    w_in_c = nc.dram_tensor("w_in_c", [1, 3072], BF16, kind="ExternalInput")
    b_in = nc.dram_tensor("b_in", [3072, 1], FP32, kind="ExternalInput")
    diags = nc.dram_tensor("diags", [128, NXP * KC * 128], BF16,
                           kind="ExternalInput")
    conv_b = nc.dram_tensor("conv_b", [Di, 1], FP32, kind="ExternalInput")
    w_xp = nc.dram_tensor("w_xp", [Di, 96], BF16, kind="ExternalInput")
    w_dt = nc.dram_tensor("w_dt", [R, DiS], BF16, kind="ExternalInput")
    b_dt = nc.dram_tensor("b_dt", [DiS, 1], FP32, kind="ExternalInput")
    a_mat = nc.dram_tensor("a_mat", [DiS, N], FP32, kind="ExternalInput")
    d_vec = nc.dram_tensor("d_vec", [DiS, 1], FP32, kind="ExternalInput")
    w_out = nc.dram_tensor("w_out", [DiS, Dm], BF16, kind="ExternalInput")
    eye = nc.dram_tensor("eye", [2 * N, 2 * N * 128], BF16,
                         kind="ExternalInput")
    ident = nc.dram_tensor("ident", [128, 128], BF16, kind="ExternalInput")
    p_out = nc.dram_tensor("p_out", [Dm, L], BF16, kind="ExternalOutput")

    with tile.TileContext(nc) as tc:
        with (
            tc.tile_pool(name="pers", bufs=1) as pers,
            tc.tile_pool(name="hpers", bufs=1) as hp,
            tc.tile_pool(name="wks", bufs=3) as wks,
            tc.tile_pool(name="cvt", bufs=2) as cvt,
            tc.tile_pool(name="mega", bufs=2) as mega,
            tc.tile_pool(name="reps", bufs=1) as repsp,
            tc.tile_pool(name="pss", bufs=2, space="PSUM") as pss,
            tc.tile_pool(name="psin", bufs=1, space="PSUM") as psin,
            tc.tile_pool(name="psy", bufs=2, space="PSUM") as psy,
        ):
            # ---------------- persistent small tiles ----------------
            # DMA issue order matters for warmup: x-stats + in_proj biases
            # first, scan-phase constants deferred via late_loads().
            ident_sb = pers.tile([128, 128], BF16, name="identsb",
                                 tag="identsb")
            wxa = pers.tile([128, NXP * 96], BF16, name="wxa", tag="wxa")
            wdt = pers.tile([R, DiS], BF16, name="wdt", tag="wdt")
            a_all = pers.tile([128, NSH * N], FP32, name="aal", tag="aal")
            d_all = pers.tile([128, NSH], FP32, name="dal", tag="dal")
            bdt_all = pers.tile([128, NSH], FP32, name="bdal", tag="bdal")
            cb_all = pers.tile([128, NXP], FP32, name="cbal", tag="cbal")
            bin_all = pers.tile([128, 24], FP32, name="bial", tag="bial")
            wx = [wxa[:, i * 96:(i + 1) * 96] for i in range(NXP)]
            a_sb = [a_all[:, i * N:(i + 1) * N] for i in range(NSH)]
            d_sb = [d_all[:, i:i + 1] for i in range(NSH)]
            bdt_sb = [bdt_all[:, i:i + 1] for i in range(NSH)]
            cb_sb = [cb_all[:, i:i + 1] for i in range(NXP)]
            bin_sb = [bin_all[:, i:i + 1] for i in range(24)]

            def bias_loads():
                nc.sync.dma_start(
                    bin_all[:],
                    b_in.ap().rearrange("(k p) c -> p (k c)", p=128))
                nc.sync.dma_start(
                    cb_all[:],
                    conv_b.ap().rearrange("(k p) c -> p (k c)", p=128))
                nc.sync.dma_start(ident_sb[:], ident.ap())

            eye_sb = pers.tile([2 * N, 2 * N * 128], BF16, name="eyesb",
                               tag="eyesb")

            def late_loads():
                nc.sync.dma_start(eye_sb[:], eye.ap())
                nc.sync.dma_start(
                    wxa[:].rearrange("p (k c) -> p k c", c=96),
                    w_xp.ap().rearrange("(k p) c -> p k c", p=128))
                nc.sync.dma_start(wdt[:], w_dt.ap())
                nc.sync.dma_start(
                    a_all[:].rearrange("p (k c) -> p k c", c=N),
                    a_mat.ap().rearrange("(k p) c -> p k c", p=128))
                nc.sync.dma_start(
                    d_all[:],
                    d_vec.ap().rearrange("(k p) c -> p (k c)", p=128))
                nc.sync.dma_start(
                    bdt_all[:],
                    b_dt.ap().rearrange("(k p) c -> p (k c)", p=128))

            binh_all = pers.tile([128, 24], FP32, name="biha", tag="biha")
            cbh_all = pers.tile([128, NXP], FP32, name="cbha", tag="cbha")
            binh_sb = [binh_all[:, i:i + 1] for i in range(24)]
            cbh_sb = [cbh_all[:, i:i + 1] for i in range(NXP)]

            def half_biases():
                nc.scalar.mul(binh_all[:], bin_all[:], 0.5)
                nc.scalar.mul(cbh_all[:], cb_all[:], 0.5)

            tails = [pers.tile([128, 3], BF16, name=f"tl{i}", tag=f"tl{i}")
                     for i in range(NXP)]
            carry = [[pers.tile([128, SEG], FP32, name=f"cr{m}_{nb}",
                                tag=f"cr{m}_{nb}") for nb in range(4)]
                     for m in range(NSH)]

            sz = [[hp.tile([128, HL], BF16, name=f"sz{m}_{h}",
                           tag=f"sz{m}_{h}") for h in range(2)]
                  for m in range(NSH)]
            w_t = [[hp.tile([128, HL], BF16, name=f"wt{m}_{h}",
                            tag=f"wt{m}_{h}") for h in range(2)]
                   for m in range(NSH)]
            deltaT = [[hp.tile([128, HL], BF16, name=f"dl{m}_{h}",
                               tag=f"dl{m}_{h}") for h in range(2)]
                      for m in range(NSH)]
            xpd = [[hp.tile([128, HL], BF16, name=f"xd{m}_{h}",
                            tag=f"xd{m}_{h}") for h in range(2)]
                   for m in range(NSH)]

            xp_cur = [None] * NXP
            z1_cur = [None] * NMD

            def scr():
                return pss.tile([128, HL], FP32, name="scr", tag="scr",
                                bufs=2)

            # ---------------- phase builders ----------------
            def stats(h):
                cs = slice(h * HL, (h + 1) * HL)
                rstd_b = cvt.tile([128, HL], BF16, name="rstdb", tag="rstdb",
                                  bufs=1)
                nc.sync.dma_start(rstd_b[:], rstd_in.ap()[:, cs])
                mr_row = cvt.tile([1, HL], BF16, name="mr", tag="mr", bufs=1)
                nc.sync.dma_start(mr_row[:], mr_in.ap()[:, cs])
                z1a = cvt.tile([128, NMD * HL], BF16, name="z1a", tag="z1a",
                               bufs=1)
                nc.sync.dma_start(
                    z1a[:].rearrange("p (k c) -> p k c", c=HL),
                    xT.ap()[:, cs].rearrange("(k p) c -> p k c", p=128))
                nc.vector.tensor_tensor(
                    z1a[:].rearrange("p (k c) -> p k c", c=HL),
                    z1a[:].rearrange("p (k c) -> p k c", c=HL),
                    rstd_b[:].unsqueeze(1).broadcast_to([128, NMD, HL]),
                    OP.mult)
                for i in range(NMD):
                    z1_cur[i] = z1a[:, i * HL:(i + 1) * HL]
                return mr_row

            def ip_group(h, g, mr_row):
                mts = [g * 2, g * 2 + 1]
                pst = [psin.tile([128, HL], FP32, name=f"psi{j}",
                                 tag=f"psi{j}", bufs=2) for j in range(2)]
                for kt in range(NMD):
                    wkt = wks.tile([128, 256], BF16, name="wkt", tag="wkt",
                                   bufs=8)
                    nc.sync.dma_start(
                        wkt[:], w_in.ap()[kt * 128:(kt + 1) * 128,
                                          g * 256:(g + 1) * 256])
                    for j in range(2):
                        nc.tensor.matmul(pst[j][:],
                                         wkt[:, j * 128:(j + 1) * 128],
                                         z1_cur[kt][:],
                                         start=(kt == 0), stop=False)
                wcg = wks.tile([1, 256], BF16, name="wcg", tag="wcg",
                               bufs=3)
                nc.sync.dma_start(wcg[:],
                                  w_in_c.ap()[:, g * 256:(g + 1) * 256])
                for j in range(2):
                    nc.tensor.matmul(pst[j][:],
                                     wcg[:, j * 128:(j + 1) * 128],
                                     mr_row[:], start=False, stop=True)
                for j in range(2):
                    mt = mts[j]
                    if mt < NXP:
                        xpad = cvt.tile([128, HL + 3], BF16, name="xpad",
                                        tag="xpad", bufs=3)
                        if h == 0:
                            nc.vector.memset(xpad[:, 0:3], 0.0)
                        else:
                            nc.scalar.copy(xpad[:, 0:3], tails[mt][:])
                        nc.scalar.activation(xpad[:, 3:3 + HL], pst[j][:],
                                             AF.Identity, bias=bin_sb[mt])
                        if h == 0:
                            nc.scalar.copy(tails[mt][:], xpad[:, HL:HL + 3])
                        dgt = cvt.tile([128, KC * 128], BF16, name="dgt",
                                       tag="dgt", bufs=2)
                        nc.sync.dma_start(
                            dgt[:], diags.ap()[:, mt * KC * 128:
                                               (mt + 1) * KC * 128])
                        pcv = scr()
                        for c in range(KC):
                            nc.tensor.matmul(pcv[:],
                                             dgt[:, c * 128:(c + 1) * 128],
                                             xpad[:, c:c + HL],
                                             start=(c == 0),
                                             stop=(c == KC - 1))
                        xc = cvt.tile([128, HL], BF16, name="xc", tag="xc",
                                      bufs=2)
                        nc.scalar.activation(xc[:], pcv[:], AF.Identity,
                                             bias=cb_sb[mt])
                        th = cvt.tile([128, HL], BF16, name="th", tag="th",
                                      bufs=2)
                        nc.scalar.activation(th[:], pcv[:], AF.Tanh,
                                             bias=cbh_sb[mt], scale=0.5)
                        sg = cvt.tile([128, HL], BF16, name="sg", tag="sg",
                                      bufs=2)
                        nc.vector.tensor_scalar(sg[:], th[:], 0.5, 0.5,
                                                OP.mult, OP.add)
                        xp_cur[mt] = cvt.tile([128, HL], BF16,
                                              name=f"xp{mt}", tag=f"xp{mt}",
                                              bufs=1)
                        nc.vector.tensor_tensor(xp_cur[mt][:], xc[:], sg[:],
                                                OP.mult)
                    else:
                        zx = cvt.tile([128, HL], BF16, name="zx", tag="zx",
                                      bufs=2)
                        nc.scalar.activation(zx[:], pst[j][:], AF.Identity,
                                             bias=bin_sb[mt])
                        th = cvt.tile([128, HL], BF16, name="th", tag="th",
                                      bufs=2)
                        nc.scalar.activation(th[:], pst[j][:], AF.Tanh,
                                             bias=binh_sb[mt], scale=0.5)
                        sg = cvt.tile([128, HL], BF16, name="sg", tag="sg",
                                      bufs=2)
                        nc.vector.tensor_scalar(sg[:], th[:], 0.5, 0.5,
                                                OP.mult, OP.add)
                        nc.vector.tensor_tensor(sz[mt - NXP][h][:], zx[:],
                                                sg[:], OP.mult)

            def xd(h, mts=range(NSH)):
                psd = None
                if 0 in mts:
                    psd = scr()
                if 0 in mts:
                    for kt in range(NXP):
                        nc.tensor.matmul(psd[0:96, :], wx[kt],
                                         xp_cur[kt][:], start=(kt == 0),
                                         stop=(kt == NXP - 1))
                    dbl = cvt.tile([96, HL], BF16, name="dbl", tag="dbl",
                                   bufs=1)
                    nc.scalar.copy(dbl[:], psd[0:96, :])
                    bc_pack = cvt.tile([2 * N, HL], BF16, name="bcp",
                                       tag="bcp", bufs=1)
                    nc.scalar.copy(bc_pack[:], dbl[R:R + 2 * N, :])
                    xd.dbl, xd.bcp = dbl, bc_pack
                dbl, bc_pack = xd.dbl, xd.bcp
                for mt in mts:
                    psdt = scr()
                    nc.tensor.matmul(psdt[:],
                                     wdt[:, mt * 128:(mt + 1) * 128],
                                     dbl[0:R, :], start=True, stop=True)
                    ue = cvt.tile([128, HL], BF16, name="ue", tag="ue",
                                  bufs=2)
                    nc.scalar.activation(ue[:], psdt[:], AF.Exp,
                                         bias=bdt_sb[mt])
                    us = cvt.tile([128, HL], BF16, name="us", tag="us",
                                  bufs=2)
                    nc.vector.tensor_scalar(us[:], ue[:], -0.5, 1.0,
                                            OP.mult, OP.add)
                    nc.vector.tensor_tensor(deltaT[mt][h][:], ue[:], us[:],
                                            OP.mult)
                    nc.vector.tensor_tensor(w_t[mt][h][:], deltaT[mt][h][:],
                                            xp_cur[mt][:], OP.mult)
                    nc.vector.tensor_scalar_mul(xpd[mt][h][:], xp_cur[mt][:],
                                                 d_sb[mt])
                return bc_pack

            def build_reps(h, bc_pack):
                out = []
                for nb in range(4):
                    pair = []
                    for which in range(2):  # 0: B rows, 1: C rows
                        rep = repsp.tile([128, MW], BF16,
                                         name=f"rp{which}{nb}",
                                         tag=f"rp{which}{nb}", bufs=1)
                        for j in range(SEG):
                            n = nb * SEG + j
                            row = which * N + n
                            psb = scr()
                            nc.tensor.matmul(
                                psb[:], eye_sb[:, row * 128:(row + 1) * 128],
                                bc_pack[:], start=True, stop=True)
                            nc.scalar.copy(rep[:, j * HL:(j + 1) * HL],
                                           psb[:])
                        pair.append(rep)
                    out.append(pair)
                return out

            def sc_mt(h, mt, reps, interleave, pre_yf=()):
                y_ps = psy.tile([128, HL], FP32, name="yps", tag="yps",
                                bufs=2)
                nc.tensor.matmul(y_ps[:], ident_sb[:], xpd[mt][h][:],
                                 start=True, stop=False)
                pend_carry = [None]
                for nb in range(4):
                    brep, crep = reps[nb][0], reps[nb][1]
                    dA = mega.tile([128, MW], BF16, name="dA", tag="dA",
                                   bufs=3)
                    for j in range(SEG):
                        n = nb * SEG + j
                        nc.scalar.activation(dA[:, j * HL:(j + 1) * HL],
                                             deltaT[mt][h][:], AF.Exp,
                                             scale=a_sb[mt][:, n:n + 1])
                    dBx = mega.tile([128, MW], BF16, name="dBx", tag="dBx",
                                    bufs=2)
                    wrep = w_t[mt][h][:].unsqueeze(1).broadcast_to(
                        [128, SEG, HL])
                    brv = brep[:].rearrange("p (s q) -> p s q", q=HL)
                    dxv = dBx[:].rearrange("p (s q) -> p s q", q=HL)
                    nc.vector.tensor_tensor(dxv, wrep, brv, OP.mult)
                    dAv = dA[:].rearrange("p (s q) -> p s q", q=HL)
                    if h == 1:
                        tmp = cvt.tile([128, SEG], FP32, name="fixt",
                                       tag="fixt", bufs=2)
                        nc.vector.tensor_tensor(tmp[:], dAv[:, :, 0],
                                                carry[mt][nb][:], OP.mult)
                        nc.vector.tensor_tensor(dxv[:, :, 0], dxv[:, :, 0],
                                                tmp[:], OP.add)
                    nc.vector.memset(dAv[:, :, 0:1], 0.0)
                    hh = mega.tile([128, MW], BF16, name="hh", tag="hh",
                                   bufs=2)
                    nc.vector.tensor_tensor_scan(hh[:], dA[:], dBx[:], 0.0,
                                                 OP.mult, OP.add)
                    if h == 0:
                        if pend_carry[0] is not None:
                            pnb, phh = pend_carry[0]
                            nc.scalar.copy(carry[mt][pnb][:],
                                           phh[:, :, HL - 1])
                        pend_carry[0] = (
                            nb, hh[:].rearrange("p (s q) -> p s q", q=HL))
                    nc.vector.tensor_tensor(dA[:], hh[:], crep[:],
                                            OP.mult)
                    for j in range(SEG):
                        nc.tensor.matmul(y_ps[:], ident_sb[:],
                                         dA[:, j * HL:(j + 1) * HL],
                                         start=False,
                                         stop=(nb == 3 and j == SEG - 1))
                if h == 0 and pend_carry[0] is not None:
                    pnb, phh = pend_carry[0]
                    nc.scalar.copy(carry[mt][pnb][:], phh[:, :, HL - 1])
                for fn in pre_yf:
                    fn()
                nc.vector.tensor_tensor(sz[mt][h][:], sz[mt][h][:], y_ps[:],
                                        OP.mult)
                for fn in interleave:
                    fn()

            def fn_group(h, mo):
                pso = scr()
                for kt in range(NSH):
                    wob = wks.tile([128, 128], BF16, name="wob", tag="wob",
                                   bufs=3)
                    nc.sync.dma_start(
                        wob[:], w_out.ap()[kt * 128:(kt + 1) * 128,
                                           mo * 128:(mo + 1) * 128])
                    nc.tensor.matmul(pso[:], wob[:], sz[kt][h][:],
                                     start=(kt == 0), stop=(kt == NSH - 1))
                osb = cvt.tile([128, HL], BF16, name="osb", tag="osb",
                               bufs=1)
                nc.scalar.copy(osb[:], pso[:])
                nc.sync.dma_start(
                    p_out.ap()[mo * 128:(mo + 1) * 128,
                               h * HL:(h + 1) * HL], osb[:])

            # h1 out_proj split: kt 0..5 accumulated early (passA, overlapped
            # with the last h1 scans), kt 6..7 + combine in a short tail.
            pA = [pers.tile([128, HL], BF16, name=f"pA{mo}", tag=f"pA{mo}")
                  for mo in range(NMD)]


            def fn1_a(mo):
                pso = scr()
                for kt in range(6):
                    wob = wks.tile([128, 128], BF16, name="wob", tag="wob",
                                   bufs=3)
                    nc.sync.dma_start(
                        wob[:], w_out.ap()[kt * 128:(kt + 1) * 128,
                                           mo * 128:(mo + 1) * 128])
                    nc.tensor.matmul(pso[:], wob[:], sz[kt][1][:],
                                     start=(kt == 0), stop=(kt == 5))
                nc.scalar.copy(pA[mo][:], pso[:])

            def fn1_b(mo):
                pso = scr()
                for kt in range(6, NSH):
                    wob = wks.tile([128, 128], BF16, name="wob", tag="wob",
                                   bufs=3)
                    nc.sync.dma_start(
                        wob[:], w_out.ap()[kt * 128:(kt + 1) * 128,
                                           mo * 128:(mo + 1) * 128])
                    nc.tensor.matmul(pso[:], wob[:], sz[kt][1][:],
                                     start=(kt == 6), stop=(kt == NSH - 1))
                osb = cvt.tile([128, HL], BF16, name="osb", tag="osb",
                               bufs=1)
                nc.vector.tensor_tensor(osb[:], pso[:], pA[mo][:], OP.add)
                nc.sync.dma_start(
                    p_out.ap()[mo * 128:(mo + 1) * 128,
                               HL:2 * HL], osb[:])

            def chunk(units, nslots, at=2):
                out = [[] for _ in range(nslots)]
                out[at] = list(units)
                return out

            # ---------------- emission ----------------
            mr0 = stats(0)
            bias_loads()
            half_biases()
            ip_group(0, 0, mr0)
            late_loads()
            for g in range(1, 8):
                ip_group(0, g, mr0)
            bcp0 = xd(0)
            reps0 = build_reps(0, bcp0)

            def mk_ip0(g):
                return lambda: ip_group(0, g, mr0)

            mr1_box = [None]
            reps1_box = [None]

            def u_stats1():
                mr1_box[0] = stats(1)

            def mk_ip1(g):
                return lambda: ip_group(1, g, mr1_box[0])

            bcp1_box = [None]

            def u_xd1a():
                bcp1_box[0] = xd(1, mts=range(0, 4))

            def u_xd1b():
                xd(1, mts=range(4, NSH))

            def u_reps1():
                reps1_box[0] = build_reps(1, bcp1_box[0])

            g = [mk_ip1(i) for i in range(12)]
            z0 = [mk_ip0(i) for i in range(8, 12)]
            sched0 = [[], [u_stats1, g[0], g[1]], [g[2], g[3], g[4]],
                      [g[5], g[6], g[7]], [g[8], g[9]],
                      [g[10], g[11], u_xd1a], [u_xd1b], [u_reps1]]
            pre0 = [[z0[0], z0[1]], [z0[2], z0[3]], [], [], [], [], [], []]
            for mt in range(NSH):
                sc_mt(0, mt, reps0, sched0[mt], pre0[mt])

            reps1 = reps1_box[0]

            f = [lambda mo=mo: fn_group(0, mo) for mo in range(NMD)]
            fa = [lambda mo=mo: fn1_a(mo) for mo in range(NMD)]
            sched1 = [[f[0], f[1]], [f[2], f[3]], [f[4], f[5]],
                      [f[6], f[7]], [],
                      [fa[0], fa[1], fa[2], fa[3]],
                      [fa[4], fa[5], fa[6], fa[7]], []]
            for mt in range(NSH):
                sc_mt(1, mt, reps1, sched1[mt])

            for mo in range(NMD):
                fn1_b(mo)

    nc.compile()
    return nc


# ----------------------------------------------------------------- launch 2
MH = Dm // 2  # 512 output rows per core


def build_launch2():
    nc = bacc.Bacc("TRN2", target_bir_lowering=False, debug=False,
                   num_devices=NCORES)
    xbt = nc.dram_tensor("xbt", [Dm, HL], BF16, kind="ExternalInput")
    pf = nc.dram_tensor("pf", [Dm, HL], BF16, kind="ExternalInput")
    pb = nc.dram_tensor("pb", [Dm, HL], BF16, kind="ExternalInput")
    wg = nc.dram_tensor("wg", [2 * Dm, MH], BF16, kind="ExternalInput")
    wv = nc.dram_tensor("wv", [2 * Dm, MH], BF16, kind="ExternalInput")
    b_of = nc.dram_tensor("b_of", [Dm, 1], FP32, kind="ExternalInput")
    b_ob = nc.dram_tensor("b_ob", [Dm, 1], FP32, kind="ExternalInput")
    bg = nc.dram_tensor("bg", [MH, 1], FP32, kind="ExternalInput")
    bv = nc.dram_tensor("bv", [MH, 1], FP32, kind="ExternalInput")
    ot = nc.dram_tensor("ot", [MH, HL], FP32, kind="ExternalOutput")

    NMH = MH // 128  # 4

    with tile.TileContext(nc) as tc:
        with (
            tc.tile_pool(name="pers", bufs=1) as pers,
            tc.tile_pool(name="tx", bufs=3) as txp,
            tc.tile_pool(name="ps", bufs=2, space="PSUM") as psp,
        ):
            wgt_all = pers.tile([128, 16 * MH], BF16, name="wga",
                                tag="wga")
            wvt_all = pers.tile([128, 16 * MH], BF16, name="wva",
                                tag="wva")
            xb_all = pers.tile([128, NMD * HL], BF16, name="xba", tag="xba")
            pf_all = pers.tile([128, NMD * HL], BF16, name="pfa", tag="pfa")
            pb_all = pers.tile([128, NMD * HL], BF16, name="pba", tag="pba")
            bo_fa = pers.tile([128, NMD], FP32, name="bofa", tag="bofa")
            bo_ba = pers.tile([128, NMD], FP32, name="boba", tag="boba")
            nc.sync.dma_start(bo_fa[:],
                              b_of.ap().rearrange("(k p) c -> p (k c)",
                                                  p=128))
            nc.sync.dma_start(bo_ba[:],
                              b_ob.ap().rearrange("(k p) c -> p (k c)",
                                                  p=128))
            for t, h in ((xb_all, xbt), (pf_all, pf), (pb_all, pb)):
                for hf in range(2):
                    cs = slice(hf * 4 * HL, (hf + 1) * 4 * HL)
                    rs = slice(hf * 512, (hf + 1) * 512)
                    nc.sync.dma_start(
                        t[:, cs].rearrange("p (k c) -> p k c", c=HL),
                        h.ap()[rs, :].rearrange("(k p) c -> p k c", p=128))
            for b4 in range(4):
                cs = slice(b4 * 4 * MH, (b4 + 1) * 4 * MH)
                rs = slice(b4 * 512, (b4 + 1) * 512)
                nc.sync.dma_start(
                    wgt_all[:, cs].rearrange("p (k c) -> p k c", c=MH),
                    wg.ap()[rs, :].rearrange("(k p) c -> p k c", p=128))
                nc.sync.dma_start(
                    wvt_all[:, cs].rearrange("p (k c) -> p k c", c=MH),
                    wv.ap()[rs, :].rearrange("(k p) c -> p k c", p=128))

            fwd_bf = [pers.tile([128, HL], BF16, name=f"fb{i}", tag=f"fb{i}")
                      for i in range(NMD)]
            bwd_bf = [pers.tile([128, HL], BF16, name=f"bb{i}", tag=f"bb{i}")
                      for i in range(NMD)]
            for i in range(NMD):
                cs = slice(i * HL, (i + 1) * HL)
                nc.vector.scalar_tensor_tensor(
                    fwd_bf[i][:], xb_all[:, cs], bo_fa[:, i:i + 1],
                    pf_all[:, cs], OP.add, OP.add)
                nc.vector.scalar_tensor_tensor(
                    bwd_bf[i][:], xb_all[:, cs], bo_ba[:, i:i + 1],
                    pb_all[:, cs], OP.add, OP.add)

            psg4 = [psp.tile([128, HL], FP32, name=f"psg{mo}",
                             tag=f"psg{mo}", bufs=1) for mo in range(NMH)]
            psv4 = [psp.tile([128, HL], FP32, name=f"psv{mo}",
                             tag=f"psv{mo}", bufs=1) for mo in range(NMH)]
            def wsl(t, i, mo):
                return t[:, i * MH + mo * 128: i * MH + (mo + 1) * 128]

            bg_a = pers.tile([128, NMH], FP32, name="bga", tag="bga")
            bv_a = pers.tile([128, NMH], FP32, name="bva", tag="bva")
            nc.sync.dma_start(bg_a[:],
                              bg.ap().rearrange("(k p) c -> p (k c)", p=128))
            nc.sync.dma_start(bv_a[:],
                              bv.ap().rearrange("(k p) c -> p (k c)", p=128))

            def emit_gate(mo):
                sl = slice(mo * 128, (mo + 1) * 128)
                bgt = bg_a[:, mo:mo + 1]
                bvt = bv_a[:, mo:mo + 1]
                ssum = txp.tile([128, HL], FP32, name="ss", tag="ss")
                nc.vector.tensor_tensor(ssum[:], fwd_bf[mo][:],
                                        bwd_bf[mo][:], OP.add)
                g = txp.tile([128, HL], FP32, name="g", tag="g")
                v = txp.tile([128, HL], FP32, name="v", tag="v")
                nc.scalar.activation(g[:], psg4[mo][:], AF.Sigmoid,
                                     bias=bgt)
                nc.scalar.activation(v[:], psv4[mo][:], AF.Identity,
                                     bias=bvt)
                d = txp.tile([128, HL], FP32, name="dd", tag="dd")
                nc.vector.tensor_tensor(d[:], v[:], ssum[:], OP.subtract)
                m = txp.tile([128, HL], FP32, name="mm", tag="mm")
                nc.vector.tensor_tensor(m[:], g[:], d[:], OP.mult)
                o = txp.tile([128, HL], FP32, name="oo", tag="oo")
                nc.vector.tensor_tensor(o[:], m[:], ssum[:], OP.add)
                o2 = txp.tile([128, HL], FP32, name="o2", tag="o2")
                nc.scalar.mul(o2[:], o[:], 0.5)
                nc.sync.dma_start(ot.ap()[sl, :], o2[:])

            for mo in range(NMH):
                for i in range(NMD):
                    nc.tensor.matmul(psg4[mo][:], wsl(wgt_all, i, mo),
                                     fwd_bf[i][:], start=(i == 0),
                                     stop=False)
                    nc.tensor.matmul(psg4[mo][:], wsl(wgt_all, 8 + i, mo),
                                     bwd_bf[i][:], start=False,
                                     stop=(i == NMD - 1))
                    nc.tensor.matmul(psv4[mo][:], wsl(wvt_all, i, mo),
                                     fwd_bf[i][:], start=(i == 0),
                                     stop=False)
                    nc.tensor.matmul(psv4[mo][:], wsl(wvt_all, 8 + i, mo),
                                     bwd_bf[i][:], start=False,
                                     stop=(i == NMD - 1))
                emit_gate(mo)

    nc.compile()
    return nc


# ------------------------------------------------------------------- host
_cache = {}


def _get_nc(which):
    if which not in _cache:
        _cache[which] = build_launch1() if which == 1 else build_launch2()
    return _cache[which]


def prep_launch1_inmaps(x, ln_w, ln_b, W_in, b_in, conv_w, conv_b, W_xproj,
                        W_dt, b_dt, A_log, D, W_out, b_out):
    in_maps = []
    eye = np.zeros((2 * N, 2 * N * 128), np.float32)
    for j in range(2 * N):
        eye[j, j * 128:(j + 1) * 128] = 1.0
    eye = eye.astype(BF)
    ident_np = np.eye(128, dtype=np.float32).astype(BF)
    xf = [np.ascontiguousarray(x[b].T) for b in range(B)]
    xr = [np.ascontiguousarray(x[b, ::-1].T) for b in range(B)]
    idx = np.arange(128)
    for core in range(NCORES):
        b, dr, s = core >> 2, (core >> 1) & 1, core & 1
        sl = slice(s * DiS, (s + 1) * DiS)
        W_eff = ln_w[dr][:, None] * W_in[dr]
        b_eff = ln_b[dr] @ W_in[dr] + b_in[dr]
        perm = np.concatenate([np.arange(s * DiS, (s + 1) * DiS),
                               np.arange((1 - s) * DiS, (2 - s) * DiS)])
        cols = np.concatenate([perm, Di + s * DiS + np.arange(DiS)])
        Wc = W_eff[:, cols]
        cwp = conv_w[dr][perm]
        dg = np.zeros((128, NXP * KC * 128), np.float32)
        for mt in range(NXP):
            for j in range(KC):
                dg[idx, (mt * KC + j) * 128 + idx] = cwp[mt * 128 + idx, j]
        xTc = (xf if dr == 0 else xr)[b]
        mu_r = xTc.mean(axis=0)
        var_r = xTc.var(axis=0)
        rstd_r = (1.0 / np.sqrt(var_r + EPS)).astype(np.float32)
        in_maps.append({
            "xT": xTc.astype(BF),
            "rstd_in": np.broadcast_to(
                rstd_r[None, :], (128, L)).astype(BF),
            "mr_in": (mu_r * rstd_r)[None, :].astype(BF),
            "w_in": Wc.astype(BF),
            "w_in_c": (-Wc.sum(0, keepdims=True)).astype(BF),
            "b_in": b_eff[cols][:, None].astype(np.float32),
            "diags": dg.astype(BF),
            "conv_b": conv_b[dr][perm][:, None].astype(np.float32),
            "w_xp": W_xproj[dr][perm].astype(BF),
            "w_dt": W_dt[dr][:, sl].astype(BF),
            "b_dt": b_dt[dr][sl][:, None].astype(np.float32),
            "a_mat": (-np.exp(A_log[dr][sl])).astype(np.float32),
            "d_vec": D[dr][sl][:, None].astype(np.float32),
            "w_out": W_out[dr][sl, :].astype(BF),
            "eye": eye,
            "ident": ident_np,
        })
    return in_maps, xf


def prep_launch2_inmaps(res1, xf, Wg, bg, Wv, bv, b_out):
    idx = lambda b, dr, s: (b << 2) | (dr << 1) | s
    pf = [[np.asarray(res1[idx(b, 0, s)]["p_out"], np.float32)
           for s in range(2)] for b in range(B)]
    pb = [[np.asarray(res1[idx(b, 1, s)]["p_out"], np.float32)[:, ::-1]
           for s in range(2)] for b in range(B)]
    in_maps = []
    for core in range(NCORES):
        b, mh, lh = core >> 2, (core >> 1) & 1, core & 1
        cs = slice(lh * HL, (lh + 1) * HL)
        ms = slice(mh * MH, (mh + 1) * MH)
        # row permutation: own m-half rows first
        p2 = np.concatenate([np.arange(mh * MH, (mh + 1) * MH),
                             np.arange((1 - mh) * MH, (2 - mh) * MH)])
        wrows = np.concatenate([p2, Dm + p2])
        in_maps.append({
            "xbt": np.ascontiguousarray(xf[b][p2][:, cs]).astype(BF),
            "pf": np.ascontiguousarray(
                (pf[b][0] + pf[b][1])[p2][:, cs]).astype(BF),
            "pb": np.ascontiguousarray(
                (pb[b][0] + pb[b][1])[p2][:, cs]).astype(BF),
            "wg": np.ascontiguousarray(Wg[wrows][:, ms]).astype(BF),
            "wv": np.ascontiguousarray(Wv[wrows][:, ms]).astype(BF),
            "b_of": b_out[0][p2][:, None].astype(np.float32),
            "b_ob": b_out[1][p2][:, None].astype(np.float32),
            "bg": bg[ms][:, None].astype(np.float32),
            "bv": bv[ms][:, None].astype(np.float32),
        })
    return in_maps


def kernel(x, ln_w, ln_b, W_in, b_in, conv_w, conv_b, W_xproj, W_dt, b_dt,
           A_log, D, W_out, b_out, Wg, bg, Wv, bv):
    x = np.asarray(x, np.float32)
    args = [np.asarray(a, np.float32) for a in
            (ln_w, ln_b, W_in, b_in, conv_w, conv_b, W_xproj, W_dt, b_dt,
             A_log, D, W_out, b_out)]
    Wg, bg, Wv, bv = (np.asarray(a, np.float32) for a in (Wg, bg, Wv, bv))

    in1, xf = prep_launch1_inmaps(x, *args)
    nc1 = _get_nc(1)
    res1 = run_bass_kernel_spmd(nc1, in1, core_ids=list(range(NCORES))).results

    in2 = prep_launch2_inmaps(res1, xf, Wg, bg, Wv, bv, args[-1])
    nc2 = _get_nc(2)
    res2 = run_bass_kernel_spmd(nc2, in2, core_ids=list(range(NCORES))).results

    out = np.empty((B, L, Dm), np.float32)
    for core in range(NCORES):
        b, mh, lh = core >> 2, (core >> 1) & 1, core & 1
        out[b, lh * HL:(lh + 1) * HL, mh * MH:(mh + 1) * MH] = \
            res2[core]["ot"].T
    return out



# revision 45
# speedup vs baseline: 1.0047x; 1.0047x over previous
"""BiMambaBlock Trainium2 kernel (8-core SPMD via Bass/Tile), v3.

Launch1 core = (b, dir, s) with s = half of d_inner; the kernel is split
into two time-halves (h0, h1) with instruction emission interleaved so
the PE-heavy in_proj of h1 overlaps the DVE-bound selective scan of h0.
The 16 scan states are packed as 8x512 segments into [128, 2048] mega
tiles (4 n-blocks); cross-half scan state is carried via a dBx
boundary-column fixup with zeroed dA segment starts.

v3 notes (measured on HW): all big elementwise streams live on DVE
(bf16 packed -> 2x mode; GpSimd shares SBUF ports with DVE and its TT
is 3.3x slower per element, so offloading there slows the scan);
activations stay within the ACT exp_and_others table set (tanh-trick
silu, no sigmoid) to avoid ACT_TABLE_LOAD thrash; persistent constants
are loaded via a handful of batched rearranged DMAs (sync-queue issue
cost ~0.6us each dominates small transfers); x ships as bf16; the h1
out_proj is split kt 0..5 / 6..7 so only a short tail trails the last
scan.

Launch2 core = (b, m-half, L-half) gated combine; host permutes rows so
every core's own m-half sits in block 0 (uniform SPMD program).
"""

import os
import sys

sys.path.insert(0, "/opt/trn_rl_repo")

import numpy as np
import ml_dtypes

import concourse.bass as bass
import concourse.mybir as mybir
import concourse.tile as tile
from concourse import bacc
from concourse.bass_utils import run_bass_kernel_spmd

FP32 = mybir.dt.float32
BF16 = mybir.dt.bfloat16
AF = mybir.ActivationFunctionType
OP = mybir.AluOpType
BF = ml_dtypes.bfloat16

B, L, Dm, Di, N, R, KC = 2, 1024, 1024, 2048, 16, 64, 4
DiS = Di // 2
EPS = 1e-5
NCORES = 8
HL = L // 2          # 512 per half
SEG = 4              # segments per n-block
MW = SEG * HL        # 2048 mega width
NXP = Di // 128      # 16
NSH = DiS // 128     # 8
NMD = Dm // 128      # 8


# ----------------------------------------------------------------- launch 1
def build_launch1():
    nc = bacc.Bacc("TRN2", target_bir_lowering=False, debug=False,
                   num_devices=NCORES)
    xT = nc.dram_tensor("xT", [Dm, L], BF16, kind="ExternalInput")
    rstd_in = nc.dram_tensor("rstd_in", [128, L], BF16, kind="ExternalInput")
    mr_in = nc.dram_tensor("mr_in", [1, L], BF16, kind="ExternalInput")
    w_in = nc.dram_tensor("w_in", [Dm, 3072], BF16, kind="ExternalInput")
    w_in_c = nc.dram_tensor("w_in_c", [1, 3072], BF16, kind="ExternalInput")
    b_in = nc.dram_tensor("b_in", [3072, 1], FP32, kind="ExternalInput")
    diags = nc.dram_tensor("diags", [128, NXP * KC * 128], BF16,
                           kind="ExternalInput")
    conv_b = nc.dram_tensor("conv_b", [Di, 1], FP32, kind="ExternalInput")
    w_xp = nc.dram_tensor("w_xp", [Di, 96], BF16, kind="ExternalInput")
    w_dt = nc.dram_tensor("w_dt", [R, DiS], BF16, kind="ExternalInput")
    b_dt = nc.dram_tensor("b_dt", [DiS, 1], FP32, kind="ExternalInput")
    a_mat = nc.dram_tensor("a_mat", [DiS, N], FP32, kind="ExternalInput")
    d_vec = nc.dram_tensor("d_vec", [DiS, 1], FP32, kind="ExternalInput")
    w_out = nc.dram_tensor("w_out", [DiS, Dm], BF16, kind="ExternalInput")
    eye = nc.dram_tensor("eye", [2 * N, 2 * N * 128], BF16,
                         kind="ExternalInput")
    ident = nc.dram_tensor("ident", [128, 128], BF16, kind="ExternalInput")
    p_out = nc.dram_tensor("p_out", [Dm, L], BF16, kind="ExternalOutput")

    with tile.TileContext(nc) as tc:
        with (
            tc.tile_pool(name="pers", bufs=1) as pers,
            tc.tile_pool(name="hpers", bufs=1) as hp,
            tc.tile_pool(name="wks", bufs=3) as wks,
            tc.tile_pool(name="cvt", bufs=2) as cvt,
            tc.tile_pool(name="mega", bufs=2) as mega,
            tc.tile_pool(name="reps", bufs=1) as repsp,
            tc.tile_pool(name="pss", bufs=2, space="PSUM") as pss,
            tc.tile_pool(name="psin", bufs=1, space="PSUM") as psin,
            tc.tile_pool(name="psy", bufs=2, space="PSUM") as psy,
        ):
            # ---------------- persistent small tiles ----------------
            # DMA issue order matters for warmup: x-stats + in_proj biases
            # first, scan-phase constants deferred via late_loads().
            ident_sb = pers.tile([128, 128], BF16, name="identsb",
                                 tag="identsb")
            wxa = pers.tile([128, NXP * 96], BF16, name="wxa", tag="wxa")
            wdt = pers.tile([R, DiS], BF16, name="wdt", tag="wdt")
            a_all = pers.tile([128, NSH * N], FP32, name="aal", tag="aal")
            d_all = pers.tile([128, NSH], FP32, name="dal", tag="dal")
            bdt_all = pers.tile([128, NSH], FP32, name="bdal", tag="bdal")
            cb_all = pers.tile([128, NXP], FP32, name="cbal", tag="cbal")
            bin_all = pers.tile([128, 24], FP32, name="bial", tag="bial")
            wx = [wxa[:, i * 96:(i + 1) * 96] for i in range(NXP)]
            a_sb = [a_all[:, i * N:(i + 1) * N] for i in range(NSH)]
            d_sb = [d_all[:, i:i + 1] for i in range(NSH)]
            bdt_sb = [bdt_all[:, i:i + 1] for i in range(NSH)]
            cb_sb = [cb_all[:, i:i + 1] for i in range(NXP)]
            bin_sb = [bin_all[:, i:i + 1] for i in range(24)]

            def bias_loads():
                nc.sync.dma_start(
                    bin_all[:],
                    b_in.ap().rearrange("(k p) c -> p (k c)", p=128))
                nc.sync.dma_start(
                    cb_all[:],
                    conv_b.ap().rearrange("(k p) c -> p (k c)", p=128))
                nc.sync.dma_start(ident_sb[:], ident.ap())

            eye_sb = pers.tile([2 * N, 2 * N * 128], BF16, name="eyesb",
                               tag="eyesb")

            def late_loads():
                nc.sync.dma_start(eye_sb[:], eye.ap())
                nc.sync.dma_start(
                    wxa[:].rearrange("p (k c) -> p k c", c=96),
                    w_xp.ap().rearrange("(k p) c -> p k c", p=128))
                nc.sync.dma_start(wdt[:], w_dt.ap())
                nc.sync.dma_start(
                    a_all[:].rearrange("p (k c) -> p k c", c=N),
                    a_mat.ap().rearrange("(k p) c -> p k c", p=128))
                nc.sync.dma_start(
                    d_all[:],
                    d_vec.ap().rearrange("(k p) c -> p (k c)", p=128))
                nc.sync.dma_start(
                    bdt_all[:],
                    b_dt.ap().rearrange("(k p) c -> p (k c)", p=128))

            binh_all = pers.tile([128, 24], FP32, name="biha", tag="biha")
            cbh_all = pers.tile([128, NXP], FP32, name="cbha", tag="cbha")
            binh_sb = [binh_all[:, i:i + 1] for i in range(24)]
            cbh_sb = [cbh_all[:, i:i + 1] for i in range(NXP)]

            def half_biases():
                nc.scalar.mul(binh_all[:], bin_all[:], 0.5)
                nc.scalar.mul(cbh_all[:], cb_all[:], 0.5)

            tails = [pers.tile([128, 3], BF16, name=f"tl{i}", tag=f"tl{i}")
                     for i in range(NXP)]
            carry = [[pers.tile([128, SEG], FP32, name=f"cr{m}_{nb}",
                                tag=f"cr{m}_{nb}") for nb in range(4)]
                     for m in range(NSH)]

            sz = [[hp.tile([128, HL], BF16, name=f"sz{m}_{h}",
                           tag=f"sz{m}_{h}") for h in range(2)]
                  for m in range(NSH)]
            w_t = [[hp.tile([128, HL], BF16, name=f"wt{m}_{h}",
                            tag=f"wt{m}_{h}") for h in range(2)]
                   for m in range(NSH)]
            deltaT = [[hp.tile([128, HL], BF16, name=f"dl{m}_{h}",
                               tag=f"dl{m}_{h}") for h in range(2)]
                      for m in range(NSH)]
            xpd = [[hp.tile([128, HL], BF16, name=f"xd{m}_{h}",
                            tag=f"xd{m}_{h}") for h in range(2)]
                   for m in range(NSH)]

            xp_cur = [None] * NXP
            z1_cur = [None] * NMD

            def scr():
                return pss.tile([128, HL], FP32, name="scr", tag="scr",
                                bufs=2)

            # ---------------- phase builders ----------------
            def stats(h):
                cs = slice(h * HL, (h + 1) * HL)
                rstd_b = cvt.tile([128, HL], BF16, name="rstdb", tag="rstdb",
                                  bufs=1)
                nc.sync.dma_start(rstd_b[:], rstd_in.ap()[:, cs])
                mr_row = cvt.tile([1, HL], BF16, name="mr", tag="mr", bufs=1)
                nc.sync.dma_start(mr_row[:], mr_in.ap()[:, cs])
                z1a = cvt.tile([128, NMD * HL], BF16, name="z1a", tag="z1a",
                               bufs=1)
                nc.sync.dma_start(
                    z1a[:].rearrange("p (k c) -> p k c", c=HL),
                    xT.ap()[:, cs].rearrange("(k p) c -> p k c", p=128))
                nc.vector.tensor_tensor(
                    z1a[:].rearrange("p (k c) -> p k c", c=HL),
                    z1a[:].rearrange("p (k c) -> p k c", c=HL),
                    rstd_b[:].unsqueeze(1).broadcast_to([128, NMD, HL]),
                    OP.mult)
                for i in range(NMD):
                    z1_cur[i] = z1a[:, i * HL:(i + 1) * HL]
                return mr_row

            def ip_group(h, g, mr_row):
                mts = [g * 2, g * 2 + 1]
                pst = [psin.tile([128, HL], FP32, name=f"psi{j}",
                                 tag=f"psi{j}", bufs=2) for j in range(2)]
                for kt in range(NMD):
                    wkt = wks.tile([128, 256], BF16, name="wkt", tag="wkt",
                                   bufs=8)
                    nc.sync.dma_start(
                        wkt[:], w_in.ap()[kt * 128:(kt + 1) * 128,
                                          g * 256:(g + 1) * 256])
                    for j in range(2):
                        nc.tensor.matmul(pst[j][:],
                                         wkt[:, j * 128:(j + 1) * 128],
                                         z1_cur[kt][:],
                                         start=(kt == 0), stop=False)
                wcg = wks.tile([1, 256], BF16, name="wcg", tag="wcg",
                               bufs=3)
                nc.sync.dma_start(wcg[:],
                                  w_in_c.ap()[:, g * 256:(g + 1) * 256])
                for j in range(2):
                    nc.tensor.matmul(pst[j][:],
                                     wcg[:, j * 128:(j + 1) * 128],
                                     mr_row[:], start=False, stop=True)
                for j in range(2):
                    mt = mts[j]
                    if mt < NXP:
                        xpad = cvt.tile([128, HL + 3], BF16, name="xpad",
                                        tag="xpad", bufs=3)
                        if h == 0:
                            nc.vector.memset(xpad[:, 0:3], 0.0)
                        else:
                            nc.scalar.copy(xpad[:, 0:3], tails[mt][:])
                        nc.scalar.activation(xpad[:, 3:3 + HL], pst[j][:],
                                             AF.Identity, bias=bin_sb[mt])
                        if h == 0:
                            nc.scalar.copy(tails[mt][:], xpad[:, HL:HL + 3])
                        dgt = cvt.tile([128, KC * 128], BF16, name="dgt",
                                       tag="dgt", bufs=2)
                        nc.sync.dma_start(
                            dgt[:], diags.ap()[:, mt * KC * 128:
                                               (mt + 1) * KC * 128])
                        pcv = scr()
                        for c in range(KC):
                            nc.tensor.matmul(pcv[:],
                                             dgt[:, c * 128:(c + 1) * 128],
                                             xpad[:, c:c + HL],
                                             start=(c == 0),
                                             stop=(c == KC - 1))
                        xc = cvt.tile([128, HL], BF16, name="xc", tag="xc",
                                      bufs=2)
                        nc.scalar.activation(xc[:], pcv[:], AF.Identity,
                                             bias=cb_sb[mt])
                        th = cvt.tile([128, HL], BF16, name="th", tag="th",
                                      bufs=2)
                        nc.scalar.activation(th[:], pcv[:], AF.Tanh,
                                             bias=cbh_sb[mt], scale=0.5)
                        sg = cvt.tile([128, HL], BF16, name="sg", tag="sg",
                                      bufs=2)
                        nc.vector.tensor_scalar(sg[:], th[:], 0.5, 0.5,
                                                OP.mult, OP.add)
                        xp_cur[mt] = cvt.tile([128, HL], BF16,
                                              name=f"xp{mt}", tag=f"xp{mt}",
                                              bufs=1)
                        nc.vector.tensor_tensor(xp_cur[mt][:], xc[:], sg[:],
                                                OP.mult)
                    else:
                        zx = cvt.tile([128, HL], BF16, name="zx", tag="zx",
                                      bufs=2)
                        nc.scalar.activation(zx[:], pst[j][:], AF.Identity,
                                             bias=bin_sb[mt])
                        th = cvt.tile([128, HL], BF16, name="th", tag="th",
                                      bufs=2)
                        nc.scalar.activation(th[:], pst[j][:], AF.Tanh,
                                             bias=binh_sb[mt], scale=0.5)
                        sg = cvt.tile([128, HL], BF16, name="sg", tag="sg",
                                      bufs=2)
                        nc.vector.tensor_scalar(sg[:], th[:], 0.5, 0.5,
                                                OP.mult, OP.add)
                        nc.vector.tensor_tensor(sz[mt - NXP][h][:], zx[:],
                                                sg[:], OP.mult)

            def xd(h, mts=range(NSH)):
                psd = None
                if 0 in mts:
                    psd = scr()
                if 0 in mts:
                    for kt in range(NXP):
                        nc.tensor.matmul(psd[0:96, :], wx[kt],
                                         xp_cur[kt][:], start=(kt == 0),
                                         stop=(kt == NXP - 1))
                    dbl = cvt.tile([96, HL], BF16, name="dbl", tag="dbl",
                                   bufs=1)
                    nc.scalar.copy(dbl[:], psd[0:96, :])
                    bc_pack = cvt.tile([2 * N, HL], BF16, name="bcp",
                                       tag="bcp", bufs=1)
                    nc.scalar.copy(bc_pack[:], dbl[R:R + 2 * N, :])
                    xd.dbl, xd.bcp = dbl, bc_pack
                dbl, bc_pack = xd.dbl, xd.bcp
                for mt in mts:
                    psdt = scr()
                    nc.tensor.matmul(psdt[:],
                                     wdt[:, mt * 128:(mt + 1) * 128],
                                     dbl[0:R, :], start=True, stop=True)
                    ue = cvt.tile([128, HL], BF16, name="ue", tag="ue",
                                  bufs=2)
                    nc.scalar.activation(ue[:], psdt[:], AF.Exp,
                                         bias=bdt_sb[mt])
                    us = cvt.tile([128, HL], BF16, name="us", tag="us",
                                  bufs=2)
                    nc.vector.tensor_scalar(us[:], ue[:], -0.5, 1.0,
                                            OP.mult, OP.add)
                    nc.vector.tensor_tensor(deltaT[mt][h][:], ue[:], us[:],
                                            OP.mult)
                    nc.vector.tensor_tensor(w_t[mt][h][:], deltaT[mt][h][:],
                                            xp_cur[mt][:], OP.mult)
                    nc.vector.tensor_scalar_mul(xpd[mt][h][:], xp_cur[mt][:],
                                                 d_sb[mt])
                return bc_pack

            def build_reps(h, bc_pack):
                out = []
                for nb in range(4):
                    pair = []
                    for which in range(2):  # 0: B rows, 1: C rows
                        rep = repsp.tile([128, MW], BF16,
                                         name=f"rp{which}{nb}",
                                         tag=f"rp{which}{nb}", bufs=1)
                        for j in range(SEG):
                            n = nb * SEG + j
                            row = which * N + n
                            psb = scr()
                            nc.tensor.matmul(
                                psb[:], eye_sb[:, row * 128:(row + 1) * 128],
                                bc_pack[:], start=True, stop=True)
                            nc.scalar.copy(rep[:, j * HL:(j + 1) * HL],
                                           psb[:])
                        pair.append(rep)
                    out.append(pair)
                return out

            def sc_mt(h, mt, reps, interleave, pre_yf=()):
                y_ps = psy.tile([128, HL], FP32, name="yps", tag="yps",
                                bufs=2)
                nc.tensor.matmul(y_ps[:], ident_sb[:], xpd[mt][h][:],
                                 start=True, stop=False)
                pend_carry = [None]
                for nb in range(4):
                    brep, crep = reps[nb][0], reps[nb][1]
                    dA = mega.tile([128, MW], BF16, name="dA", tag="dA",
                                   bufs=3)
                    for j in range(SEG):
                        n = nb * SEG + j
                        nc.scalar.activation(dA[:, j * HL:(j + 1) * HL],
                                             deltaT[mt][h][:], AF.Exp,
                                             scale=a_sb[mt][:, n:n + 1])
                    dBx = mega.tile([128, MW], BF16, name="dBx", tag="dBx",
                                    bufs=2)
                    wrep = w_t[mt][h][:].unsqueeze(1).broadcast_to(
                        [128, SEG, HL])
                    brv = brep[:].rearrange("p (s q) -> p s q", q=HL)
                    dxv = dBx[:].rearrange("p (s q) -> p s q", q=HL)
                    nc.vector.tensor_tensor(dxv, wrep, brv, OP.mult)
                    dAv = dA[:].rearrange("p (s q) -> p s q", q=HL)
                    if h == 1:
                        tmp = cvt.tile([128, SEG], FP32, name="fixt",
                                       tag="fixt", bufs=2)
                        nc.vector.tensor_tensor(tmp[:], dAv[:, :, 0],
                                                carry[mt][nb][:], OP.mult)
                        nc.vector.tensor_tensor(dxv[:, :, 0], dxv[:, :, 0],
                                                tmp[:], OP.add)
                    nc.vector.memset(dAv[:, :, 0:1], 0.0)
                    hh = mega.tile([128, MW], BF16, name="hh", tag="hh",
                                   bufs=2)
                    nc.vector.tensor_tensor_scan(hh[:], dA[:], dBx[:], 0.0,
                                                 OP.mult, OP.add)
                    if h == 0:
                        if pend_carry[0] is not None:
                            pnb, phh = pend_carry[0]
                            nc.scalar.copy(carry[mt][pnb][:],
                                           phh[:, :, HL - 1])
                        pend_carry[0] = (
                            nb, hh[:].rearrange("p (s q) -> p s q", q=HL))
                    nc.vector.tensor_tensor(dA[:], hh[:], crep[:],
                                            OP.mult)
                    for j in range(SEG):
                        nc.tensor.matmul(y_ps[:], ident_sb[:],
                                         dA[:, j * HL:(j + 1) * HL],
                                         start=False,
                                         stop=(nb == 3 and j == SEG - 1))
                if h == 0 and pend_carry[0] is not None:
                    pnb, phh = pend_carry[0]
                    nc.scalar.copy(carry[mt][pnb][:], phh[:, :, HL - 1])
                for fn in pre_yf:
                    fn()
                nc.vector.tensor_tensor(sz[mt][h][:], sz[mt][h][:], y_ps[:],
                                        OP.mult)
                for fn in interleave:
                    fn()

            def fn_group(h, mo):
                pso = scr()
                for kt in range(NSH):
                    wob = wks.tile([128, 128], BF16, name="wob", tag="wob",
                                   bufs=3)
                    nc.sync.dma_start(
                        wob[:], w_out.ap()[kt * 128:(kt + 1) * 128,
                                           mo * 128:(mo + 1) * 128])
                    nc.tensor.matmul(pso[:], wob[:], sz[kt][h][:],
                                     start=(kt == 0), stop=(kt == NSH - 1))
                osb = cvt.tile([128, HL], BF16, name="osb", tag="osb",
                               bufs=1)
                nc.scalar.copy(osb[:], pso[:])
                nc.sync.dma_start(
                    p_out.ap()[mo * 128:(mo + 1) * 128,
                               h * HL:(h + 1) * HL], osb[:])

            # h1 out_proj split: kt 0..5 accumulated early (passA, overlapped
            # with the last h1 scans), kt 6..7 + combine in a short tail.
            pA = [pers.tile([128, HL], BF16, name=f"pA{mo}", tag=f"pA{mo}")
                  for mo in range(NMD)]


            def fn1_a(mo):
                pso = scr()
                for kt in range(6):
                    wob = wks.tile([128, 128], BF16, name="wob", tag="wob",
                                   bufs=3)
                    nc.sync.dma_start(
                        wob[:], w_out.ap()[kt * 128:(kt + 1) * 128,
                                           mo * 128:(mo + 1) * 128])
                    nc.tensor.matmul(pso[:], wob[:], sz[kt][1][:],
                                     start=(kt == 0), stop=(kt == 5))
                nc.scalar.copy(pA[mo][:], pso[:])

            def fn1_b(mo):
                pso = scr()
                for kt in range(6, NSH):
                    wob = wks.tile([128, 128], BF16, name="wob", tag="wob",
                                   bufs=3)
                    nc.sync.dma_start(
                        wob[:], w_out.ap()[kt * 128:(kt + 1) * 128,
                                           mo * 128:(mo + 1) * 128])
                    nc.tensor.matmul(pso[:], wob[:], sz[kt][1][:],
                                     start=(kt == 6), stop=(kt == NSH - 1))
                osb = cvt.tile([128, HL], BF16, name="osb", tag="osb",
                               bufs=1)
                nc.vector.tensor_tensor(osb[:], pso[:], pA[mo][:], OP.add)
                nc.sync.dma_start(
                    p_out.ap()[mo * 128:(mo + 1) * 128,
                               HL:2 * HL], osb[:])

            def chunk(units, nslots, at=2):
                out = [[] for _ in range(nslots)]
                out[at] = list(units)
                return out

            # ---------------- emission ----------------
            mr0 = stats(0)
            bias_loads()
            half_biases()
            ip_group(0, 0, mr0)
            late_loads()
            for g in range(1, 8):
                ip_group(0, g, mr0)
            bcp0 = xd(0)
            reps0 = build_reps(0, bcp0)

            def mk_ip0(g):
                return lambda: ip_group(0, g, mr0)

            mr1_box = [None]
            reps1_box = [None]

            def u_stats1():
                mr1_box[0] = stats(1)

            def mk_ip1(g):
                return lambda: ip_group(1, g, mr1_box[0])

            bcp1_box = [None]

            def u_xd1a():
                bcp1_box[0] = xd(1, mts=range(0, 4))

            def u_xd1b():
                xd(1, mts=range(4, NSH))

            def u_reps1():
                reps1_box[0] = build_reps(1, bcp1_box[0])

            g = [mk_ip1(i) for i in range(12)]
            z0 = [mk_ip0(i) for i in range(8, 12)]
            sched0 = [[], [u_stats1, g[0], g[1]], [g[2], g[3], g[4]],
                      [g[5], g[6], g[7]], [g[8], g[9]],
                      [g[10], g[11], u_xd1a], [u_xd1b], [u_reps1]]
            pre0 = [[z0[0], z0[1]], [z0[2], z0[3]], [], [], [], [], [], []]
            for mt in range(NSH):
                sc_mt(0, mt, reps0, sched0[mt], pre0[mt])

            reps1 = reps1_box[0]

            f = [lambda mo=mo: fn_group(0, mo) for mo in range(NMD)]
            fa = [lambda mo=mo: fn1_a(mo) for mo in range(NMD)]
            sched1 = [[f[0], f[1]], [f[2], f[3]], [f[4], f[5]],
                      [f[6], f[7]], [],
                      [fa[0], fa[1], fa[2], fa[3]],
                      [fa[4], fa[5], fa[6], fa[7]], []]
            for mt in range(NSH):
                sc_mt(1, mt, reps1, sched1[mt])

            for mo in range(NMD):
                fn1_b(mo)

    nc.compile()
    return nc


# ----------------------------------------------------------------- launch 2
MH = Dm // 2  # 512 output rows per core


def build_launch2():
    nc = bacc.Bacc("TRN2", target_bir_lowering=False, debug=False,
                   num_devices=NCORES)
    xbt = nc.dram_tensor("xbt", [Dm, HL], BF16, kind="ExternalInput")
    pf = nc.dram_tensor("pf", [Dm, HL], BF16, kind="ExternalInput")
    pb = nc.dram_tensor("pb", [Dm, HL], BF16, kind="ExternalInput")
    wg = nc.dram_tensor("wg", [2 * Dm, MH], BF16, kind="ExternalInput")
    wv = nc.dram_tensor("wv", [2 * Dm, MH], BF16, kind="ExternalInput")
    b_of = nc.dram_tensor("b_of", [Dm, 1], FP32, kind="ExternalInput")
    b_ob = nc.dram_tensor("b_ob", [Dm, 1], FP32, kind="ExternalInput")
    bg = nc.dram_tensor("bg", [MH, 1], FP32, kind="ExternalInput")
    bv = nc.dram_tensor("bv", [MH, 1], FP32, kind="ExternalInput")
    ot = nc.dram_tensor("ot", [MH, HL], FP32, kind="ExternalOutput")

    NMH = MH // 128  # 4

    with tile.TileContext(nc) as tc:
        with (
            tc.tile_pool(name="pers", bufs=1) as pers,
            tc.tile_pool(name="tx", bufs=3) as txp,
            tc.tile_pool(name="ps", bufs=2, space="PSUM") as psp,
        ):
            wgt_all = pers.tile([128, 16 * MH], BF16, name="wga",
                                tag="wga")
            wvt_all = pers.tile([128, 16 * MH], BF16, name="wva",
                                tag="wva")
            xb_all = pers.tile([128, NMD * HL], BF16, name="xba", tag="xba")
            pf_all = pers.tile([128, NMD * HL], BF16, name="pfa", tag="pfa")
            pb_all = pers.tile([128, NMD * HL], BF16, name="pba", tag="pba")
            bo_fa = pers.tile([128, NMD], FP32, name="bofa", tag="bofa")
            bo_ba = pers.tile([128, NMD], FP32, name="boba", tag="boba")
            nc.sync.dma_start(bo_fa[:],
                              b_of.ap().rearrange("(k p) c -> p (k c)",
                                                  p=128))
            nc.sync.dma_start(bo_ba[:],
                              b_ob.ap().rearrange("(k p) c -> p (k c)",
                                                  p=128))
            for t, h in ((xb_all, xbt), (pf_all, pf), (pb_all, pb)):
                for hf in range(2):
                    cs = slice(hf * 4 * HL, (hf + 1) * 4 * HL)
                    rs = slice(hf * 512, (hf + 1) * 512)
                    nc.sync.dma_start(
                        t[:, cs].rearrange("p (k c) -> p k c", c=HL),
                        h.ap()[rs, :].rearrange("(k p) c -> p k c", p=128))
            for b4 in range(4):
                cs = slice(b4 * 4 * MH, (b4 + 1) * 4 * MH)
                rs = slice(b4 * 512, (b4 + 1) * 512)
                nc.sync.dma_start(
                    wgt_all[:, cs].rearrange("p (k c) -> p k c", c=MH),
                    wg.ap()[rs, :].rearrange("(k p) c -> p k c", p=128))
                nc.sync.dma_start(
                    wvt_all[:, cs].rearrange("p (k c) -> p k c", c=MH),
                    wv.ap()[rs, :].rearrange("(k p) c -> p k c", p=128))

            fwd_bf = [pers.tile([128, HL], BF16, name=f"fb{i}", tag=f"fb{i}")
                      for i in range(NMD)]
            bwd_bf = [pers.tile([128, HL], BF16, name=f"bb{i}", tag=f"bb{i}")
                      for i in range(NMD)]
            for i in range(NMD):
                cs = slice(i * HL, (i + 1) * HL)
                nc.vector.scalar_tensor_tensor(
                    fwd_bf[i][:], xb_all[:, cs], bo_fa[:, i:i + 1],
                    pf_all[:, cs], OP.add, OP.add)
                nc.vector.scalar_tensor_tensor(
                    bwd_bf[i][:], xb_all[:, cs], bo_ba[:, i:i + 1],
                    pb_all[:, cs], OP.add, OP.add)

            psg4 = [psp.tile([128, HL], FP32, name=f"psg{mo}",
                             tag=f"psg{mo}", bufs=1) for mo in range(NMH)]
            for _ in range(70):
                nc.tensor.matmul(psg4[3][0:NMD, 0:NMD], bo_fa[:], bo_fa[:],
                                 start=True, stop=True)
            psv4 = [psp.tile([128, HL], FP32, name=f"psv{mo}",
                             tag=f"psv{mo}", bufs=1) for mo in range(NMH)]
            def wsl(t, i, mo):
                return t[:, i * MH + mo * 128: i * MH + (mo + 1) * 128]

            bg_a = pers.tile([128, NMH], FP32, name="bga", tag="bga")
            bv_a = pers.tile([128, NMH], FP32, name="bva", tag="bva")
            nc.sync.dma_start(bg_a[:],
                              bg.ap().rearrange("(k p) c -> p (k c)", p=128))
            nc.sync.dma_start(bv_a[:],
                              bv.ap().rearrange("(k p) c -> p (k c)", p=128))

            def emit_gate(mo):
                sl = slice(mo * 128, (mo + 1) * 128)
                bgt = bg_a[:, mo:mo + 1]
                bvt = bv_a[:, mo:mo + 1]
                ssum = txp.tile([128, HL], FP32, name="ss", tag="ss")
                nc.vector.tensor_tensor(ssum[:], fwd_bf[mo][:],
                                        bwd_bf[mo][:], OP.add)
                g = txp.tile([128, HL], FP32, name="g", tag="g")
                v = txp.tile([128, HL], FP32, name="v", tag="v")
                nc.scalar.activation(g[:], psg4[mo][:], AF.Sigmoid,
                                     bias=bgt)
                nc.scalar.activation(v[:], psv4[mo][:], AF.Identity,
                                     bias=bvt)
                d = txp.tile([128, HL], FP32, name="dd", tag="dd")
                nc.vector.tensor_tensor(d[:], v[:], ssum[:], OP.subtract)
                m = txp.tile([128, HL], FP32, name="mm", tag="mm")
                nc.vector.tensor_tensor(m[:], g[:], d[:], OP.mult)
                o = txp.tile([128, HL], FP32, name="oo", tag="oo")
                nc.vector.tensor_tensor(o[:], m[:], ssum[:], OP.add)
                o2 = txp.tile([128, HL], FP32, name="o2", tag="o2")
                nc.scalar.mul(o2[:], o[:], 0.5)
                nc.sync.dma_start(ot.ap()[sl, :], o2[:])

            for mo in range(NMH):
                for i in range(NMD):
                    nc.tensor.matmul(psg4[mo][:], wsl(wgt_all, i, mo),
                                     fwd_bf[i][:], start=(i == 0),
                                     stop=False)
                    nc.tensor.matmul(psg4[mo][:], wsl(wgt_all, 8 + i, mo),
                                     bwd_bf[i][:], start=False,
                                     stop=(i == NMD - 1))
                    nc.tensor.matmul(psv4[mo][:], wsl(wvt_all, i, mo),
                                     fwd_bf[i][:], start=(i == 0),
                                     stop=False)
                    nc.tensor.matmul(psv4[mo][:], wsl(wvt_all, 8 + i, mo),
                                     bwd_bf[i][:], start=False,
                                     stop=(i == NMD - 1))
                emit_gate(mo)

    nc.compile()
    return nc


# ------------------------------------------------------------------- host
_cache = {}


def _get_nc(which):
    if which not in _cache:
        _cache[which] = build_launch1() if which == 1 else build_launch2()
    return _cache[which]


def prep_launch1_inmaps(x, ln_w, ln_b, W_in, b_in, conv_w, conv_b, W_xproj,
                        W_dt, b_dt, A_log, D, W_out, b_out):
    in_maps = []
    eye = np.zeros((2 * N, 2 * N * 128), np.float32)
    for j in range(2 * N):
        eye[j, j * 128:(j + 1) * 128] = 1.0
    eye = eye.astype(BF)
    ident_np = np.eye(128, dtype=np.float32).astype(BF)
    xf = [np.ascontiguousarray(x[b].T) for b in range(B)]
    xr = [np.ascontiguousarray(x[b, ::-1].T) for b in range(B)]
    idx = np.arange(128)
    for core in range(NCORES):
        b, dr, s = core >> 2, (core >> 1) & 1, core & 1
        sl = slice(s * DiS, (s + 1) * DiS)
        W_eff = ln_w[dr][:, None] * W_in[dr]
        b_eff = ln_b[dr] @ W_in[dr] + b_in[dr]
        perm = np.concatenate([np.arange(s * DiS, (s + 1) * DiS),
                               np.arange((1 - s) * DiS, (2 - s) * DiS)])
        cols = np.concatenate([perm, Di + s * DiS + np.arange(DiS)])
        Wc = W_eff[:, cols]
        cwp = conv_w[dr][perm]
        dg = np.zeros((128, NXP * KC * 128), np.float32)
        for mt in range(NXP):
            for j in range(KC):
                dg[idx, (mt * KC + j) * 128 + idx] = cwp[mt * 128 + idx, j]
        xTc = (xf if dr == 0 else xr)[b]
        mu_r = xTc.mean(axis=0)
        var_r = xTc.var(axis=0)
        rstd_r = (1.0 / np.sqrt(var_r + EPS)).astype(np.float32)
        in_maps.append({
            "xT": xTc.astype(BF),
            "rstd_in": np.broadcast_to(
                rstd_r[None, :], (128, L)).astype(BF),
            "mr_in": (mu_r * rstd_r)[None, :].astype(BF),
            "w_in": Wc.astype(BF),
            "w_in_c": (-Wc.sum(0, keepdims=True)).astype(BF),
            "b_in": b_eff[cols][:, None].astype(np.float32),
            "diags": dg.astype(BF),
            "conv_b": conv_b[dr][perm][:, None].astype(np.float32),
            "w_xp": W_xproj[dr][perm].astype(BF),
            "w_dt": W_dt[dr][:, sl].astype(BF),
            "b_dt": b_dt[dr][sl][:, None].astype(np.float32),
            "a_mat": (-np.exp(A_log[dr][sl])).astype(np.float32),
            "d_vec": D[dr][sl][:, None].astype(np.float32),
            "w_out": W_out[dr][sl, :].astype(BF),
            "eye": eye,
            "ident": ident_np,
        })
    return in_maps, xf


def prep_launch2_inmaps(res1, xf, Wg, bg, Wv, bv, b_out):
    idx = lambda b, dr, s: (b << 2) | (dr << 1) | s
    pf = [[np.asarray(res1[idx(b, 0, s)]["p_out"], np.float32)
           for s in range(2)] for b in range(B)]
    pb = [[np.asarray(res1[idx(b, 1, s)]["p_out"], np.float32)[:, ::-1]
           for s in range(2)] for b in range(B)]
    in_maps = []
    for core in range(NCORES):
        b, mh, lh = core >> 2, (core >> 1) & 1, core & 1
        cs = slice(lh * HL, (lh + 1) * HL)
        ms = slice(mh * MH, (mh + 1) * MH)
        # row permutation: own m-half rows first
        p2 = np.concatenate([np.arange(mh * MH, (mh + 1) * MH),
                             np.arange((1 - mh) * MH, (2 - mh) * MH)])
        wrows = np.concatenate([p2, Dm + p2])
        in_maps.append({
            "xbt": np.ascontiguousarray(xf[b][p2][:, cs]).astype(BF),
            "pf": np.ascontiguousarray(
                (pf[b][0] + pf[b][1])[p2][:, cs]).astype(BF),
            "pb": np.ascontiguousarray(
                (pb[b][0] + pb[b][1])[p2][:, cs]).astype(BF),
            "wg": np.ascontiguousarray(Wg[wrows][:, ms]).astype(BF),
            "wv": np.ascontiguousarray(Wv[wrows][:, ms]).astype(BF),
            "b_of": b_out[0][p2][:, None].astype(np.float32),
            "b_ob": b_out[1][p2][:, None].astype(np.float32),
            "bg": bg[ms][:, None].astype(np.float32),
            "bv": bv[ms][:, None].astype(np.float32),
        })
    return in_maps


def kernel(x, ln_w, ln_b, W_in, b_in, conv_w, conv_b, W_xproj, W_dt, b_dt,
           A_log, D, W_out, b_out, Wg, bg, Wv, bv):
    x = np.asarray(x, np.float32)
    args = [np.asarray(a, np.float32) for a in
            (ln_w, ln_b, W_in, b_in, conv_w, conv_b, W_xproj, W_dt, b_dt,
             A_log, D, W_out, b_out)]
    Wg, bg, Wv, bv = (np.asarray(a, np.float32) for a in (Wg, bg, Wv, bv))

    in1, xf = prep_launch1_inmaps(x, *args)
    nc1 = _get_nc(1)
    res1 = run_bass_kernel_spmd(nc1, in1, core_ids=list(range(NCORES))).results

    in2 = prep_launch2_inmaps(res1, xf, Wg, bg, Wv, bv, args[-1])
    nc2 = _get_nc(2)
    res2 = run_bass_kernel_spmd(nc2, in2, core_ids=list(range(NCORES))).results

    out = np.empty((B, L, Dm), np.float32)
    for core in range(NCORES):
        b, mh, lh = core >> 2, (core >> 1) & 1, core & 1
        out[b, lh * HL:(lh + 1) * HL, mh * MH:(mh + 1) * MH] = \
            res2[core]["ot"].T
    return out

